# revision 1
# baseline (speedup 1.0000x reference)
"""Multi-head causal attention (B=4, T=2048, D=1024, 16 heads) on 8 TRN2 cores.

Sharding: core c -> batch b = c//2, head-group g = c%2 (8 of 16 heads).
Each core computes its batch's QKV for its heads, flash-style causal
attention with scores kept transposed (S^T[k, q]) so softmax sums come
free via a ones-column appended to V, then a partial output projection
y_part = attn_local @ W_proj[rows]. Host sums the two head-group partials
per batch.

Matmul operands are fp16 (same ~11-bit mantissa as the PE's fast fp32r
mode, but 1-pass FWL weight loads); accumulation stays fp32 in PSUM.
"""

import math
from contextlib import ExitStack

import numpy as np

import concourse.bacc as bacc
import concourse.bass as bass
import concourse.mybir as mybir
import concourse.tile as tile
from concourse.bass_utils import run_bass_kernel_spmd

AF = mybir.ActivationFunctionType
F32 = mybir.dt.float32
F32R = mybir.dt.float32r
F16 = mybir.dt.float16
U16 = mybir.dt.uint16

B_FULL = 4
T_FULL = 2048
D_FULL = 1024
NH_FULL = 16
HD = 64


def build_program(T, D, HL, n_pat, blocks):
    """Build the per-core SPMD program.

    T: sequence length, D: model dim, HL: local heads, n_pat: number of
    distinct mixed-mask pattern tiles, blocks: per q-chunk list of
    (k_tile_index, pattern_index_or_None) for active score blocks.
    """
    CL = HL * HD            # local channels (q, k, or v width)
    NDT = D // 128          # d-tiles (contraction tiles for qkv matmuls)
    NTT = T // 128          # t-tiles
    QCW = min(512, T)       # q-chunk width
    NQC = T // QCW
    TPC = QCW // 128        # t-tiles per q-chunk
    NCT = CL // 128         # c-tiles for q/k/attn storage
    PCH = min(512, D)       # proj output chunk
    NPCH = D // PCH
    scale = 1.0 / math.sqrt(HD)

    nc = bacc.Bacc("TRN2", target_bir_lowering=False, debug=False)
    x = nc.dram_tensor("x", [T, D], F16, kind="ExternalInput").ap()
    wq = nc.dram_tensor("wq", [D, CL], F16, kind="ExternalInput").ap()
    wk = nc.dram_tensor("wk", [D, CL], F16, kind="ExternalInput").ap()
    wv = nc.dram_tensor("wv", [D, CL], F16, kind="ExternalInput").ap()
    bq = nc.dram_tensor("bq", [CL], F32, kind="ExternalInput").ap()
    bk = nc.dram_tensor("bk", [CL], F32, kind="ExternalInput").ap()
    bv = nc.dram_tensor("bv", [CL], F32, kind="ExternalInput").ap()
    wp = nc.dram_tensor("wp", [CL, D], F16, kind="ExternalInput").ap()
    bp = nc.dram_tensor("bp", [D], F32, kind="ExternalInput").ap()
    mp = nc.dram_tensor("mp", [max(n_pat, 1), 128, QCW], F16, kind="ExternalInput").ap()
    y = nc.dram_tensor("y", [T, D], F32, kind="ExternalOutput").ap()

    with tile.TileContext(nc) as tc, nc.allow_low_precision(
        reason="float32r tiles hold full-fp32 data; matmul rounds internally"
    ):
        with ExitStack() as octx:
            persist = octx.enter_context(tc.tile_pool(name="persist", bufs=1))
            kT = [persist.tile([128, T], F16, name=f"kT{i}", tag=f"kT{i}") for i in range(NCT)]
            # Q^T stored twice, zero-padded per head parity, so the scores
            # matmul can contract over the full 128 partitions (the f32r fast
            # path needs K > 64; zero rows kill the other head's channels).
            qTe = [persist.tile([128, T], F16, name=f"qTe{i}", tag=f"qTe{i}") for i in range(NCT)]
            qTo = [persist.tile([128, T], F16, name=f"qTo{i}", tag=f"qTo{i}") for i in range(NCT)]
            for i in range(NCT):
                nc.gpsimd.memset(qTe[i][HD:128, :], 0.0)
                nc.gpsimd.memset(qTo[i][0:HD, :], 0.0)
            # per-head stride 128 elements (256B) keeps the AV stationary
            # loads FWL-aligned; cols [65:128) of each head slot are junk.
            VSW = HL * 128
            vS = [persist.tile([128, VSW], F16, name=f"vS{i}", tag=f"vS{i}") for i in range(NTT)]
            for i in range(NTT):
                nc.gpsimd.memset(vS[i], 0.0)
                nc.gpsimd.memset(
                    vS[i].rearrange("p (h c) -> p h c", c=128)[:, :, HD:HD + 1], 1.0
                )
            attnT = [persist.tile([128, T], F16, name=f"attnT{i}", tag=f"attnT{i}") for i in range(NCT)]
            bqs = persist.tile([128, NCT], F32, name="bqs", tag="bqs")
            bks = persist.tile([128, NCT], F32, name="bks", tag="bks")
            nc.sync.dma_start(out=bqs, in_=bq.rearrange("(m p) -> p m", p=128))
            nc.sync.dma_start(out=bks, in_=bk.rearrange("(m p) -> p m", p=128))
            bvb = persist.tile([128, CL], F32, name="bvb", tag="bvb")
            nc.sync.dma_start(
                out=bvb,
                in_=bass.AP(tensor=bv.tensor, offset=bv.offset, ap=[[0, 128]] + list(bv.ap)),
            )

            # ---- Phase ABC: x^T (per chunk), V natural, Q^T/K^T ----
            with ExitStack() as actx:
                abc = actx.enter_context(tc.tile_pool(name="abc", bufs=2))
                wtp = actx.enter_context(tc.tile_pool(name="wtp", bufs=3))
                wvp = actx.enter_context(tc.tile_pool(name="wvp", bufs=1))
                psb = actx.enter_context(tc.tile_pool(name="psb", bufs=2, space="PSUM"))

                wvt = [wvp.tile([128, CL], F16, name=f"wvt{k}", tag=f"wvt{k}") for k in range(NDT)]
                for kd in range(NDT):
                    nc.sync.dma_start(out=wvt[kd], in_=wv[kd * 128:(kd + 1) * 128, :])

                for ntc in range(NQC):
                    # x^T chunk via xbar DMA transpose (x arrives as fp16)
                    xTc = abc.tile([128, NDT, QCW], F16, name="xTc", tag="xTc")
                    for dd in range(NDT):
                        nc.sync.dma_start_transpose(
                            xTc[:, dd, :],
                            x[ntc * QCW:(ntc + 1) * QCW, dd * 128:(dd + 1) * 128],
                        )
                    for tv in range(TPC):
                        tt = ntc * TPC + tv
                        pv = psb.tile([128, CL], F32, name="pv", tag="pv")
                        for dd in range(NDT):
                            nc.tensor.matmul(
                                pv,
                                lhsT=xTc[:, dd, tv * 128:(tv + 1) * 128],
                                rhs=wvt[dd],
                                start=(dd == 0),
                                stop=(dd == NDT - 1),
                            )
                        nc.vector.tensor_add(
                            vS[tt].rearrange("p (h c) -> p h c", c=128)[:, :, 0:HD],
                            pv.rearrange("p (h d) -> p h d", h=HL),
                            bvb.rearrange("p (h d) -> p h d", h=HL),
                        )
                    for mi in range(2 * NCT):
                        isq = mi < NCT
                        mc = mi % NCT
                        wsrc = wq if isq else wk
                        wt = wtp.tile([128, NDT, 128], F16, name="wt", tag="wt")
                        nc.sync.dma_start(
                            out=wt,
                            in_=wsrc[:, mc * 128:(mc + 1) * 128].rearrange("(n p) c -> p n c", p=128),
                        )
                        pb = psb.tile([128, QCW], F32, name="pb", tag="pb")
                        for dd in range(NDT):
                            nc.tensor.matmul(
                                pb,
                                lhsT=wt[:, dd, :],
                                rhs=xTc[:, dd, :],
                                start=(dd == 0),
                                stop=(dd == NDT - 1),
                            )
                        tsl = slice(ntc * QCW, (ntc + 1) * QCW)
                        if isq:
                            nc.vector.tensor_scalar_add(
                                qTe[mc][0:HD, tsl], pb[0:HD, :], bqs[0:HD, mc:mc + 1]
                            )
                            nc.vector.tensor_scalar_add(
                                qTo[mc][HD:128, tsl], pb[HD:128, :], bqs[HD:128, mc:mc + 1]
                            )
                        else:
                            nc.vector.tensor_scalar_add(
                                kT[mc][:, tsl], pb, bks[:, mc:mc + 1]
                            )

            # ---- Phase D+E fused, qc-outer ----
            # For each q-chunk: all heads run flash attention (k-tile pairs ->
            # one [128, 2*QCW] scores psum -> one Exp -> two AV accumulates),
            # then the output projection for that q-chunk's t-tiles runs
            # immediately, interleaving with the next q-chunk's attention.
            # E's psum tiles share the AV pool slots ([128, 512] f32 both).
            with ExitStack() as dctx:
                dp = dctx.enter_context(tc.tile_pool(name="dp", bufs=1))
                ptl = dctx.enter_context(tc.tile_pool(name="ptl", bufs=3))
                recp = dctx.enter_context(tc.tile_pool(name="recp", bufs=2))
                ysb = dctx.enter_context(tc.tile_pool(name="ysb", bufs=3))
                drp = dctx.enter_context(tc.tile_pool(name="drp", bufs=4, space="DRAM"))
                pss = dctx.enter_context(tc.tile_pool(name="pss", bufs=2, space="PSUM"))
                psav = dctx.enter_context(tc.tile_pool(name="psav", bufs=4, space="PSUM"))

                mts = [dp.tile([128, QCW], F16, name=f"mt{i}", tag=f"mt{i}") for i in range(n_pat)]
                for i in range(n_pat):
                    nc.sync.dma_start(out=mts[i], in_=mp[i])
                wps = [dp.tile([128, D], F16, name=f"wps{i}", tag=f"wps{i}") for i in range(NCT)]
                for cc in range(NCT):
                    nc.sync.dma_start(out=wps[cc], in_=wp[cc * 128:(cc + 1) * 128, :])
                bpb = dp.tile([128, D], F32, name="bpb", tag="bpb")
                nc.sync.dma_start(
                    out=bpb,
                    in_=bass.AP(tensor=bp.tensor, offset=bp.offset, ap=[[0, 128]] + list(bp.ap)),
                )

                REC_BATCH = 2  # heads per reciprocal batch (must be < pav bufs)

                for qc in range(NQC):
                    row = blocks[qc]
                    assert row, f"q-chunk {qc} has no active k-tiles"
                    pavs = {}
                    stag = recp.tile([128, QCW], F32, name="stag", tag="stag")
                    nc.gpsimd.memset(stag, 1.0)
                    rinv = recp.tile([128, QCW], F32, name="rinv", tag="rinv")
                    for h in range(HL):
                        mc = h // 2
                        qTp = (qTe if h % 2 == 0 else qTo)[mc]
                        pav = psav.tile([128, QCW], F32, name="pav", tag="pav")
                        pavs[h] = pav
                        for pi in range(0, len(row), 2):
                            pair = row[pi:pi + 2]
                            w = len(pair) * QCW
                            pS = pss.tile([128, 2 * QCW], F32, name="pS", tag="pS")
                            for sj, (ki, _) in enumerate(pair):
                                nc.tensor.matmul(
                                    pS[:, sj * QCW:(sj + 1) * QCW],
                                    lhsT=kT[mc][:, ki * 128:(ki + 1) * 128],
                                    rhs=qTp[:, qc * QCW:(qc + 1) * QCW],
                                    start=True,
                                    stop=True,
                                )
                            pT = ptl.tile([128, 2 * QCW], F16, name="pT", tag="pT")
                            nc.scalar.activation(pT[:, :w], pS[:, :w], AF.Exp, scale=scale)
                            for sj, (ki, pat) in enumerate(pair):
                                sl = pT[:, sj * QCW:(sj + 1) * QCW]
                                if pat is not None:
                                    kind, arg = pat
                                    if kind == "tri":
                                        # keep where (q - k) >= 0, else 0
                                        nc.gpsimd.affine_select(
                                            out=sl,
                                            in_=sl,
                                            pattern=[[1, QCW]],
                                            base=arg,
                                            channel_multiplier=-1,
                                            compare_op=mybir.AluOpType.is_ge,
                                            fill=0.0,
                                        )
                                    else:
                                        nc.gpsimd.tensor_mul(sl, sl, mts[arg])
                                nc.tensor.matmul(
                                    pav,
                                    lhsT=vS[ki][:, h * 128:h * 128 + 128],
                                    rhs=sl,
                                    start=(pi == 0 and sj == 0),
                                    stop=(pi + sj == len(row) - 1),
                                )
                        # l row -> stag (32-aligned slot per head in batch)
                        slot = h % REC_BATCH
                        nc.scalar.copy(stag[slot * 32:slot * 32 + 1, :], pav[HD:HD + 1, :])
                        if slot == REC_BATCH - 1 or h == HL - 1:
                            lo_h = h - slot
                            nc.vector.reciprocal(
                                rinv[0:slot * 32 + 1, :], stag[0:slot * 32 + 1, :]
                            )
                            for bh in range(lo_h, h + 1):
                                bslot = bh % REC_BATCH
                                scr = drp.tile([QCW], F32, name="scr", tag="scr")
                                nc.sync.dma_start(out=scr, in_=rinv[bslot * 32:bslot * 32 + 1, :])
                                rbs = recp.tile([HD, QCW], F32, name="rbs", tag="rbs")
                                nc.sync.dma_start(
                                    out=rbs,
                                    in_=bass.AP(tensor=scr.tensor, offset=scr.offset, ap=[[0, HD]] + list(scr.ap)),
                                )
                                nc.vector.tensor_mul(
                                    attnT[bh // 2][(bh % 2) * HD:(bh % 2) * HD + HD, qc * QCW:(qc + 1) * QCW],
                                    pavs[bh][0:HD, :],
                                    rbs,
                                )
                            if h != HL - 1:
                                stag = recp.tile([128, QCW], F32, name="stag", tag="stag")
                                nc.gpsimd.memset(stag, 1.0)
                                rinv = recp.tile([128, QCW], F32, name="rinv", tag="rinv")
                    # ---- projection for this q-chunk's t-tiles ----
                    for tv in range(TPC):
                        tt = qc * TPC + tv
                        yt = ysb.tile([128, D], F32, name="yt", tag="yt")
                        for nch in range(NPCH):
                            py = psav.tile([128, PCH], F32, name="py", tag="pav")
                            for cc in range(NCT):
                                nc.tensor.matmul(
                                    py,
                                    lhsT=attnT[cc][:, tt * 128:(tt + 1) * 128],
                                    rhs=wps[cc][:, nch * PCH:(nch + 1) * PCH],
                                    start=(cc == 0),
                                    stop=(cc == NCT - 1),
                                )
                            nc.vector.tensor_add(
                                yt[:, nch * PCH:(nch + 1) * PCH], py, bpb[:, nch * PCH:(nch + 1) * PCH]
                            )
                        nc.sync.dma_start(out=y[tt * 128:(tt + 1) * 128, :], in_=yt)
    nc.compile()
    return nc


def classify_mask(mask_bool, T):
    """Classify S^T blocks [k-tile 128, q-chunk 512] as skip / full / mixed.

    mask_bool: [T, T] bool, mask_bool[q, k] = attend(q -> k).
    Returns (blocks, patterns): blocks[qc] = list of (ki, pat_idx|None),
    patterns = np.ndarray [n_pat, 128, QCW] float32.
    """
    QCW = min(512, T)
    NQC = T // QCW
    NKT = T // 128
    maskT = mask_bool.T  # [k, q]
    patterns = []
    pat_index = {}
    blocks = []
    for qc in range(NQC):
        row = []
        for ki in range(NKT):
            blk = maskT[ki * 128:(ki + 1) * 128, qc * QCW:(qc + 1) * QCW]
            if not blk.any():
                continue
            if blk.all():
                row.append((ki, None))
                continue
            # tril-offset block? keep iff k <= q, i.e. p <= base + f
            base = qc * QCW - ki * 128
            p = np.arange(128)[:, None]
            f = np.arange(QCW)[None, :]
            if np.array_equal(blk, p <= base + f):
                row.append((ki, ("tri", base)))
                continue
            key = blk.tobytes()
            if key not in pat_index:
                pat_index[key] = len(patterns)
                patterns.append(blk.astype(np.float32))
            row.append((ki, ("pat", pat_index[key])))
        blocks.append(row)
    n_pat = len(patterns)
    if patterns:
        pats = np.stack(patterns)
    else:
        pats = np.zeros((1, 128, QCW), np.float32)
    return blocks, pats, n_pat


_prog_cache = {}


def _get_program(T, D, HL, mask_bool):
    key = (T, D, HL, mask_bool.tobytes())
    if key not in _prog_cache:
        blocks, pats, n_pat = classify_mask(mask_bool, T)
        nc = build_program(T, D, HL, n_pat, blocks)
        _prog_cache[key] = (nc, blocks, pats)
    return _prog_cache[key]


def kernel(x, W_qkv, b_qkv, W_proj, b_proj, mask):
    out, _ = run_attention(x, W_qkv, b_qkv, W_proj, b_proj, mask)
    return out


def run_attention(x, W_qkv, b_qkv, W_proj, b_proj, mask, trace=False):
    x = np.ascontiguousarray(np.asarray(x, dtype=np.float32))
    W_qkv = np.asarray(W_qkv, dtype=np.float32)
    b_qkv = np.asarray(b_qkv, dtype=np.float32)
    W_proj = np.asarray(W_proj, dtype=np.float32)
    b_proj = np.asarray(b_proj, dtype=np.float32)
    Bc, T, D = x.shape
    NH = NH_FULL
    HL = NH // 2  # heads per core (two head-groups)
    CL = HL * HD

    mask_bool = np.asarray(mask)[0, 0] != 0

    nc, blocks, pats = _get_program(T, D, HL, mask_bool)

    in_maps = []
    n_cores = 2 * Bc
    for c in range(n_cores):
        b, g = c // 2, c % 2
        sl = slice(g * CL, (g + 1) * CL)
        in_maps.append({
            "x": np.ascontiguousarray(x[b]).astype(np.float16),
            "wq": np.ascontiguousarray(W_qkv[:, 0 * D:1 * D][:, sl]).astype(np.float16),
            "wk": np.ascontiguousarray(W_qkv[:, 1 * D:2 * D][:, sl]).astype(np.float16),
            "wv": np.ascontiguousarray(W_qkv[:, 2 * D:3 * D][:, sl]).astype(np.float16),
            "bq": np.ascontiguousarray(b_qkv[0 * D:1 * D][sl]),
            "bk": np.ascontiguousarray(b_qkv[1 * D:2 * D][sl]),
            "bv": np.ascontiguousarray(b_qkv[2 * D:3 * D][sl]),
            "wp": np.ascontiguousarray(W_proj[sl, :]).astype(np.float16),
            "bp": b_proj if g == 0 else np.zeros_like(b_proj),
            "mp": pats.astype(np.float16),
        })

    res = run_bass_kernel_spmd(nc, in_maps, list(range(n_cores)), trace=trace)
    out = np.empty((Bc, T, D), np.float32)
    for b in range(Bc):
        out[b] = res.results[2 * b]["y"] + res.results[2 * b + 1]["y"]
    return out, res



# revision 10
# speedup vs baseline: 1.0664x; 1.0664x over previous
"""Multi-head causal attention (B=4, T=2048, D=1024, 16 heads) on 8 TRN2 cores.

Sharding: core c -> batch b = c//2, head-group g = c%2 (8 of 16 heads).
Each core computes its batch's QKV for its heads, flash-style causal
attention with scores kept transposed (S^T[k, q]) so softmax sums come
free via a ones-column appended to V, then a partial output projection
y_part = attn_local @ W_proj[rows]. Host sums the two head-group partials
per batch.

Schedule: a single fused loop over 512-wide t-chunks. Chunk ntc's QKV
matmuls run, then flash attention for q-chunk ntc (which needs K/V
chunks 0..ntc only, all available). The next chunk's QKV matmuls and the
previous chunk's output-projection matmuls are spliced into the
exp-latency gaps of the flash loop so the PE never idles waiting on the
scalar engine. Softmax normalization is deferred: per head the
unnormalized attention rows and the l-row are evacuated to SBUF
immediately (releasing PSUM), then one batched reciprocal per q-chunk +
a DRAM-bounce partition-broadcast produce 1/l, and the normalize
multiplies run on the vector engine underneath the next chunk's flash.

Matmul operands are fp16; accumulation stays fp32 in PSUM.
"""

import math
from collections import deque
from contextlib import ExitStack

import numpy as np

import concourse.bacc as bacc
import concourse.bass as bass
import concourse.mybir as mybir
import concourse.tile as tile
from concourse.bass_utils import run_bass_kernel_spmd

AF = mybir.ActivationFunctionType
F32 = mybir.dt.float32
F16 = mybir.dt.float16

B_FULL = 4
T_FULL = 2048
D_FULL = 1024
NH_FULL = 16
HD = 64


def build_program(T, D, HL, n_pat, blocks):
    """Build the per-core SPMD program.

    T: sequence length, D: model dim, HL: local heads, n_pat: number of
    distinct mixed-mask pattern tiles, blocks: per q-chunk list of
    (k_tile_index, pattern_index_or_None) for active score blocks.
    """
    CL = HL * HD            # local channels (q, k, or v width)
    NDT = D // 128          # d-tiles (contraction tiles for qkv matmuls)
    NTT = T // 128          # t-tiles
    QCW = min(512, T)       # q-chunk width
    NQC = T // QCW
    TPC = QCW // 128        # t-tiles per q-chunk
    NCT = CL // 128         # c-tiles for q/k/attn storage
    PCH = min(512, D)       # proj output chunk
    NPCH = D // PCH
    scale = 1.0 / math.sqrt(HD)

    nc = bacc.Bacc("TRN2", target_bir_lowering=False, debug=False)
    x = nc.dram_tensor("x", [T, D], F16, kind="ExternalInput").ap()
    wq = nc.dram_tensor("wq", [D, CL], F16, kind="ExternalInput").ap()
    wk = nc.dram_tensor("wk", [D, CL], F16, kind="ExternalInput").ap()
    wv = nc.dram_tensor("wv", [D, CL], F16, kind="ExternalInput").ap()
    bq = nc.dram_tensor("bq", [CL], F32, kind="ExternalInput").ap()
    bk = nc.dram_tensor("bk", [CL], F32, kind="ExternalInput").ap()
    bv = nc.dram_tensor("bv", [CL], F32, kind="ExternalInput").ap()
    wp = nc.dram_tensor("wp", [CL, D], F16, kind="ExternalInput").ap()
    bp = nc.dram_tensor("bp", [D], F32, kind="ExternalInput").ap()
    mp = nc.dram_tensor("mp", [max(n_pat, 1), 128, QCW], F16, kind="ExternalInput").ap()
    y = nc.dram_tensor("y", [T, D], F32, kind="ExternalOutput").ap()

    with tile.TileContext(nc) as tc, nc.allow_low_precision(
        reason="fp16 operands; matmul accumulates fp32 in PSUM"
    ):
        with ExitStack() as octx:
            persist = octx.enter_context(tc.tile_pool(name="persist", bufs=1))
            kT = [persist.tile([128, T], F16, name=f"kT{i}", tag=f"kT{i}") for i in range(NCT)]
            # Q^T stored twice, zero-padded per head parity, so the scores
            # matmul can contract over the full 128 partitions (zero rows
            # kill the other head's channels).
            qTe = [persist.tile([128, T], F16, name=f"qTe{i}", tag=f"qTe{i}") for i in range(NCT)]
            qTo = [persist.tile([128, T], F16, name=f"qTo{i}", tag=f"qTo{i}") for i in range(NCT)]
            for i in range(NCT):
                nc.gpsimd.memset(qTe[i][HD:128, :], 0.0)
                nc.gpsimd.memset(qTo[i][0:HD, :], 0.0)
            # per-head stride 128 elements (256B) keeps the AV stationary
            # loads FWL-aligned; col HD of each slot is the ones column
            # that makes the AV matmul emit the softmax sums l on row HD.
            VSW = HL * 128
            vS = [persist.tile([128, VSW], F16, name=f"vS{i}", tag=f"vS{i}") for i in range(NTT)]
            for i in range(NTT):
                nc.gpsimd.memset(vS[i], 0.0)
                nc.gpsimd.memset(
                    vS[i].rearrange("p (h c) -> p h c", c=128)[:, :, HD:HD + 1], 1.0
                )
            attnT = [persist.tile([128, T], F16, name=f"attnT{i}", tag=f"attnT{i}") for i in range(NCT)]

            # resident weights: [128, NDT, CL] with layout (n p) c -> p n c,
            # so [:, dd, sl] is W[dd*128:(dd+1)*128, sl].
            wvr = persist.tile([128, NDT, CL], F16, name="wvr", tag="wvr")
            nc.sync.dma_start(out=wvr, in_=wv.rearrange("(n p) c -> p n c", p=128))
            wqr = persist.tile([128, NDT, CL], F16, name="wqr", tag="wqr")
            nc.sync.dma_start(out=wqr, in_=wq.rearrange("(n p) c -> p n c", p=128))
            wkr = persist.tile([128, NDT, CL], F16, name="wkr", tag="wkr")
            nc.sync.dma_start(out=wkr, in_=wk.rearrange("(n p) c -> p n c", p=128))

            bqs = persist.tile([128, NCT], F32, name="bqs", tag="bqs")
            bks = persist.tile([128, NCT], F32, name="bks", tag="bks")
            nc.sync.dma_start(out=bqs, in_=bq.rearrange("(m p) -> p m", p=128))
            nc.sync.dma_start(out=bks, in_=bk.rearrange("(m p) -> p m", p=128))
            bvb = persist.tile([128, CL], F32, name="bvb", tag="bvb")
            nc.sync.dma_start(
                out=bvb,
                in_=bass.AP(tensor=bv.tensor, offset=bv.offset, ap=[[0, 128]] + list(bv.ap)),
            )
            wps = [persist.tile([128, D], F16, name=f"wps{i}", tag=f"wps{i}") for i in range(NCT)]
            for cc in range(NCT):
                nc.sync.dma_start(out=wps[cc], in_=wp[cc * 128:(cc + 1) * 128, :])
            bpb = persist.tile([128, D], F32, name="bpb", tag="bpb")
            nc.sync.dma_start(
                out=bpb,
                in_=bass.AP(tensor=bp.tensor, offset=bp.offset, ap=[[0, 128]] + list(bp.ap)),
            )
            mts = [persist.tile([128, QCW], F16, name=f"mt{i}", tag=f"mt{i}") for i in range(n_pat)]
            for i in range(n_pat):
                nc.sync.dma_start(out=mts[i], in_=mp[i])

            xtp = octx.enter_context(tc.tile_pool(name="xtp", bufs=2))
            pab = octx.enter_context(tc.tile_pool(name="pab", bufs=2, space="PSUM"))
            pss = octx.enter_context(tc.tile_pool(name="pss", bufs=2, space="PSUM"))
            psav = octx.enter_context(tc.tile_pool(name="psav", bufs=2, space="PSUM"))
            ptl = octx.enter_context(tc.tile_pool(name="ptl", bufs=3))
            utp = octx.enter_context(tc.tile_pool(name="utp", bufs=HL + 1))
            lsp = octx.enter_context(tc.tile_pool(name="lsp", bufs=HL + 1))
            ltp = octx.enter_context(tc.tile_pool(name="ltp", bufs=2))
            rvp = octx.enter_context(tc.tile_pool(name="rvp", bufs=2))
            rbp = octx.enter_context(tc.tile_pool(name="rbp", bufs=3))
            ysb = octx.enter_context(tc.tile_pool(name="ysb", bufs=2))
            drp = octx.enter_context(tc.tile_pool(name="drp", bufs=4, space="DRAM"))

            # ---- feeder: QKV compute for one chunk, as small PE items ----
            def issue_xT(ntc):
                xTc = xtp.tile([128, NDT, QCW], F16, name="xTc", tag="xTc")
                for dd in range(NDT):
                    nc.sync.dma_start_transpose(
                        xTc[:, dd, :],
                        x[ntc * QCW:(ntc + 1) * QCW, dd * 128:(dd + 1) * 128],
                    )
                return xTc

            def abc_feeder(ntc):
                """Return a deque of zero-arg callables issuing chunk ntc's
                QKV matmuls in ~2-MM items. x^T transposes issued now."""
                xTc = issue_xT(ntc)
                tsl = slice(ntc * QCW, (ntc + 1) * QCW)
                items = deque()
                for tv in range(TPC):
                    tt = ntc * TPC + tv
                    box = {}
                    for dd0 in range(0, NDT, 2):
                        def v_item(dd0=dd0, box=box, tv=tv, tt=tt):
                            if dd0 == 0:
                                box["pv"] = pab.tile([128, CL], F32, name="pv", tag="pab")
                            for dd in (dd0, dd0 + 1):
                                nc.tensor.matmul(
                                    box["pv"],
                                    lhsT=xTc[:, dd, tv * 128:(tv + 1) * 128],
                                    rhs=wvr[:, dd, :],
                                    start=(dd == 0),
                                    stop=(dd == NDT - 1),
                                )
                            if dd0 == NDT - 2:
                                nc.vector.tensor_add(
                                    vS[tt].rearrange("p (h c) -> p h c", c=128)[:, :, 0:HD],
                                    box["pv"].rearrange("p (h d) -> p h d", h=HL),
                                    bvb.rearrange("p (h d) -> p h d", h=HL),
                                )
                        items.append(v_item)
                for mi in range(2 * NCT):
                    isq = mi < NCT
                    mc = mi % NCT
                    wsrc = wqr if isq else wkr
                    box = {}
                    for dd0 in range(0, NDT, 2):
                        def qk_item(dd0=dd0, box=box, mc=mc, isq=isq, wsrc=wsrc):
                            if dd0 == 0:
                                box["pb"] = pab.tile([128, QCW], F32, name="pb", tag="pab")
                            for dd in (dd0, dd0 + 1):
                                nc.tensor.matmul(
                                    box["pb"],
                                    lhsT=wsrc[:, dd, mc * 128:(mc + 1) * 128],
                                    rhs=xTc[:, dd, :],
                                    start=(dd == 0),
                                    stop=(dd == NDT - 1),
                                )
                            if dd0 == NDT - 2:
                                pb = box["pb"]
                                if isq:
                                    nc.vector.tensor_scalar_add(
                                        qTe[mc][0:HD, tsl], pb[0:HD, :], bqs[0:HD, mc:mc + 1]
                                    )
                                    nc.vector.tensor_scalar_add(
                                        qTo[mc][HD:128, tsl], pb[HD:128, :], bqs[HD:128, mc:mc + 1]
                                    )
                                else:
                                    nc.vector.tensor_scalar_add(
                                        kT[mc][:, tsl], pb, bks[:, mc:mc + 1]
                                    )
                        items.append(qk_item)
                return items

            def proj_feeder(qc):
                """Output projection for q-chunk qc (reads normalized attnT)."""
                items = deque()
                for tv in range(TPC):
                    tt = qc * TPC + tv
                    box = {}
                    for nch in range(NPCH):
                        for cc0 in range(0, NCT, 2):
                            def p_item(cc0=cc0, nch=nch, box=box, tt=tt):
                                if nch == 0 and cc0 == 0:
                                    box["yt"] = ysb.tile([128, D], F32, name="yt", tag="yt")
                                if cc0 == 0:
                                    box["py"] = pab.tile([128, PCH], F32, name="py", tag="pab")
                                for cc in (cc0, cc0 + 1):
                                    nc.tensor.matmul(
                                        box["py"],
                                        lhsT=attnT[cc][:, tt * 128:(tt + 1) * 128],
                                        rhs=wps[cc][:, nch * PCH:(nch + 1) * PCH],
                                        start=(cc == 0),
                                        stop=(cc == NCT - 1),
                                    )
                                if cc0 == NCT - 2:
                                    nc.vector.tensor_add(
                                        box["yt"][:, nch * PCH:(nch + 1) * PCH],
                                        box["py"],
                                        bpb[:, nch * PCH:(nch + 1) * PCH],
                                    )
                                    if nch == NPCH - 1:
                                        nc.sync.dma_start(
                                            out=y[tt * 128:(tt + 1) * 128, :], in_=box["yt"]
                                        )
                            items.append(p_item)
                return items

            abc_q = deque()
            proj_q = deque()

            def pump(n):
                for _ in range(n):
                    if abc_q:
                        abc_q.popleft()()
                    elif proj_q:
                        proj_q.popleft()()
                    else:
                        return

            def drain_abc():
                while abc_q:
                    abc_q.popleft()()

            # ---- flash attention for one q-chunk ----
            def flash(qc):
                row = blocks[qc]
                assert row, f"q-chunk {qc} has no active k-tiles"
                pairs = [row[i:i + 2] for i in range(0, len(row), 2)]
                # l rows go through a DRAM bounce: engine partition bases
                # must be 32-aligned, so gather the 8 per-head l rows into
                # lbuf via DMA, then reciprocal them in one batched op.
                lbuf = drp.tile([HL, QCW], F32, name="lbuf", tag="lbuf")
                uts = {}
                for h in range(HL):
                    mc = h // 2
                    qTp = (qTe if h % 2 == 0 else qTo)[mc]
                    pav = psav.tile([128, QCW], F32, name="pav", tag="pav")
                    prev = None
                    for pi, pair in enumerate(pairs):
                        w = len(pair) * QCW
                        pS = pss.tile([128, 2 * QCW], F32, name="pS", tag="pS")
                        for sj, (ki, _) in enumerate(pair):
                            nc.tensor.matmul(
                                pS[:, sj * QCW:(sj + 1) * QCW],
                                lhsT=kT[mc][:, ki * 128:(ki + 1) * 128],
                                rhs=qTp[:, qc * QCW:(qc + 1) * QCW],
                                start=True,
                                stop=True,
                            )
                        pT = ptl.tile([128, 2 * QCW], F16, name="pT", tag="pT")
                        nc.scalar.activation(pT[:, :w], pS[:, :w], AF.Exp, scale=scale)
                        for sj, (ki, pat) in enumerate(pair):
                            if pat is not None:
                                sl = pT[:, sj * QCW:(sj + 1) * QCW]
                                kind, arg = pat
                                if kind == "tri":
                                    # keep where (q - k) >= 0, else 0
                                    nc.gpsimd.affine_select(
                                        out=sl,
                                        in_=sl,
                                        pattern=[[1, QCW]],
                                        base=arg,
                                        channel_multiplier=-1,
                                        compare_op=mybir.AluOpType.is_ge,
                                        fill=0.0,
                                    )
                                else:
                                    nc.gpsimd.tensor_mul(sl, sl, mts[arg])
                        # software pipeline: AV for the previous pair issues
                        # after this pair's scores AND a couple of feeder
                        # matmuls, so its exp latency is fully covered.
                        pump(2)
                        if prev is not None:
                            issue_av(h, pav, prev, False)
                        prev = (pT, pair, pi)
                    issue_av(h, pav, prev, True)
                    # evacuate unnormalized attn rows + l row; frees the bank
                    ut = utp.tile([HD, QCW], F16, name="ut", tag="ut")
                    nc.vector.tensor_copy(ut, pav[0:HD, :])
                    stg = lsp.tile([1, QCW], F32, name="stg", tag="stg")
                    nc.vector.tensor_copy(stg, pav[HD:HD + 1, :])
                    nc.sync.dma_start(out=lbuf[h:h + 1, :], in_=stg)
                    uts[h] = ut
                    pump(2)
                return lbuf, uts

            def issue_av(h, pav, prev, is_last):
                pT, pair, pi = prev
                npairs_last = pi == 0  # single-pair rows
                for sj, (ki, _) in enumerate(pair):
                    nc.tensor.matmul(
                        pav,
                        lhsT=vS[ki][:, h * 128:h * 128 + 128],
                        rhs=pT[:, sj * QCW:(sj + 1) * QCW],
                        start=(pi == 0 and sj == 0),
                        stop=(is_last and sj == len(pair) - 1),
                    )

            def issue_norm(qc, lbuf, uts):
                lsb = ltp.tile([HL, QCW], F32, name="lsb", tag="lsb")
                nc.sync.dma_start(out=lsb, in_=lbuf)
                rinv = rvp.tile([HL, QCW], F32, name="rinv", tag="rinv")
                nc.vector.reciprocal(rinv, lsb)
                scr = drp.tile([HL, QCW], F32, name="scr", tag="scr")
                nc.sync.dma_start(out=scr, in_=rinv)
                for h in range(HL):
                    mc = h // 2
                    hh = h % 2
                    row = scr[h:h + 1, :]
                    rbs = rbp.tile([HD, QCW], F32, name="rbs", tag="rbs")
                    nc.sync.dma_start(
                        out=rbs,
                        in_=bass.AP(tensor=row.tensor, offset=row.offset, ap=[[0, HD]] + list(row.ap)[1:]),
                    )
                    nc.vector.tensor_mul(
                        attnT[mc][hh * HD:hh * HD + HD, qc * QCW:(qc + 1) * QCW],
                        uts[h],
                        rbs,
                    )

            # ---- fused main loop ----
            abc_q.extend(abc_feeder(0))
            drain_abc()
            for ntc in range(NQC):
                if ntc + 1 < NQC:
                    abc_q.extend(abc_feeder(ntc + 1))
                lbuf, uts = flash(ntc)
                drain_abc()
                issue_norm(ntc, lbuf, uts)
                proj_q.extend(proj_feeder(ntc))
            while proj_q:
                proj_q.popleft()()
    nc.compile()
    return nc


def classify_mask(mask_bool, T):
    """Classify S^T blocks [k-tile 128, q-chunk 512] as skip / full / mixed.

    mask_bool: [T, T] bool, mask_bool[q, k] = attend(q -> k).
    Returns (blocks, patterns): blocks[qc] = list of (ki, pat_idx|None),
    patterns = np.ndarray [n_pat, 128, QCW] float32.
    """
    QCW = min(512, T)
    NQC = T // QCW
    NKT = T // 128
    maskT = mask_bool.T  # [k, q]
    patterns = []
    pat_index = {}
    blocks = []
    for qc in range(NQC):
        row = []
        for ki in range(NKT):
            blk = maskT[ki * 128:(ki + 1) * 128, qc * QCW:(qc + 1) * QCW]
            if not blk.any():
                continue
            if blk.all():
                row.append((ki, None))
                continue
            # tril-offset block? keep iff k <= q, i.e. p <= base + f
            base = qc * QCW - ki * 128
            p = np.arange(128)[:, None]
            f = np.arange(QCW)[None, :]
            if np.array_equal(blk, p <= base + f):
                row.append((ki, ("tri", base)))
                continue
            key = blk.tobytes()
            if key not in pat_index:
                pat_index[key] = len(patterns)
                patterns.append(blk.astype(np.float32))
            row.append((ki, ("pat", pat_index[key])))
        blocks.append(row)
    n_pat = len(patterns)
    if patterns:
        pats = np.stack(patterns)
    else:
        pats = np.zeros((1, 128, QCW), np.float32)
    return blocks, pats, n_pat


_prog_cache = {}


def _get_program(T, D, HL, mask_bool):
    key = (T, D, HL, mask_bool.tobytes())
    if key not in _prog_cache:
        blocks, pats, n_pat = classify_mask(mask_bool, T)
        nc = build_program(T, D, HL, n_pat, blocks)
        _prog_cache[key] = (nc, blocks, pats)
    return _prog_cache[key]


def kernel(x, W_qkv, b_qkv, W_proj, b_proj, mask):
    out, _ = run_attention(x, W_qkv, b_qkv, W_proj, b_proj, mask)
    return out


def run_attention(x, W_qkv, b_qkv, W_proj, b_proj, mask, trace=False):
    x = np.ascontiguousarray(np.asarray(x, dtype=np.float32))
    W_qkv = np.asarray(W_qkv, dtype=np.float32)
    b_qkv = np.asarray(b_qkv, dtype=np.float32)
    W_proj = np.asarray(W_proj, dtype=np.float32)
    b_proj = np.asarray(b_proj, dtype=np.float32)
    Bc, T, D = x.shape
    NH = NH_FULL
    HL = NH // 2  # heads per core (two head-groups)
    CL = HL * HD

    mask_bool = np.asarray(mask)[0, 0] != 0

    nc, blocks, pats = _get_program(T, D, HL, mask_bool)

    in_maps = []
    n_cores = 2 * Bc
    for c in range(n_cores):
        b, g = c // 2, c % 2
        sl = slice(g * CL, (g + 1) * CL)
        in_maps.append({
            "x": np.ascontiguousarray(x[b]).astype(np.float16),
            "wq": np.ascontiguousarray(W_qkv[:, 0 * D:1 * D][:, sl]).astype(np.float16),
            "wk": np.ascontiguousarray(W_qkv[:, 1 * D:2 * D][:, sl]).astype(np.float16),
            "wv": np.ascontiguousarray(W_qkv[:, 2 * D:3 * D][:, sl]).astype(np.float16),
            "bq": np.ascontiguousarray(b_qkv[0 * D:1 * D][sl]),
            "bk": np.ascontiguousarray(b_qkv[1 * D:2 * D][sl]),
            "bv": np.ascontiguousarray(b_qkv[2 * D:3 * D][sl]),
            "wp": np.ascontiguousarray(W_proj[sl, :]).astype(np.float16),
            "bp": b_proj if g == 0 else np.zeros_like(b_proj),
            "mp": pats.astype(np.float16),
        })

    res = run_bass_kernel_spmd(nc, in_maps, list(range(n_cores)), trace=trace)
    out = np.empty((Bc, T, D), np.float32)
    for b in range(Bc):
        out[b] = res.results[2 * b]["y"] + res.results[2 * b + 1]["y"]
    return out, res


# revision 14
# speedup vs baseline: 1.3736x; 1.2880x over previous
"""Multi-head causal attention (B=4, T=2048, D=1024, 16 heads) on 8 TRN2 cores.

Sharding: core c -> batch b = c//2, head-group g = c%2 (8 of 16 heads).
Each core computes its batch's QKV for its heads, flash-style causal
attention with scores kept transposed (S^T[k, q]) so softmax sums come
free via a ones-column appended to V, then a partial output projection
y_part = attn_local @ W_proj[rows]. Host sums the two head-group partials
per batch.

Schedule: a single fused loop over 512-wide t-chunks. Chunk ntc's QKV
matmuls run, then flash attention for q-chunk ntc (which needs K/V
chunks 0..ntc only, all available). The next chunk's QKV matmuls and the
previous chunk's output-projection matmuls are spliced into the
exp-latency gaps of the flash loop so the PE never idles waiting on the
scalar engine. Softmax normalization is deferred: per head the
unnormalized attention rows and the l-row are evacuated to SBUF
immediately (releasing PSUM), then one batched reciprocal per q-chunk +
a DRAM-bounce partition-broadcast produce 1/l, and the normalize
multiplies run on the vector engine underneath the next chunk's flash.

Matmul operands are fp16; accumulation stays fp32 in PSUM.
"""

import math
from collections import deque
from contextlib import ExitStack

import numpy as np

import concourse.bacc as bacc
import concourse.bass as bass
import concourse.mybir as mybir
import concourse.tile as tile
from concourse.bass_utils import run_bass_kernel_spmd

AF = mybir.ActivationFunctionType
F32 = mybir.dt.float32
F16 = mybir.dt.float16

B_FULL = 4
T_FULL = 2048
D_FULL = 1024
NH_FULL = 16
HD = 64


def build_program(T, D, HL, n_pat, blocks):
    """Build the per-core SPMD program.

    T: sequence length, D: model dim, HL: local heads, n_pat: number of
    distinct mixed-mask pattern tiles, blocks: per q-chunk list of
    (k_tile_index, pattern_index_or_None) for active score blocks.
    """
    CL = HL * HD            # local channels (q, k, or v width)
    NDT = D // 128          # d-tiles (contraction tiles for qkv matmuls)
    NTT = T // 128          # t-tiles
    QCW = min(512, T)       # q-chunk width
    NQC = T // QCW
    TPC = QCW // 128        # t-tiles per q-chunk
    NCT = CL // 128         # c-tiles for q/k/attn storage
    PCH = min(512, D)       # proj output chunk
    NPCH = D // PCH
    scale = 1.0 / math.sqrt(HD)

    nc = bacc.Bacc("TRN2", target_bir_lowering=False, debug=False)
    x = nc.dram_tensor("x", [T, D], F16, kind="ExternalInput").ap()
    wq = nc.dram_tensor("wq", [D, CL], F16, kind="ExternalInput").ap()
    wk = nc.dram_tensor("wk", [D, CL], F16, kind="ExternalInput").ap()
    wv = nc.dram_tensor("wv", [D, CL], F16, kind="ExternalInput").ap()
    bq = nc.dram_tensor("bq", [CL], F32, kind="ExternalInput").ap()
    bk = nc.dram_tensor("bk", [CL], F32, kind="ExternalInput").ap()
    bv = nc.dram_tensor("bv", [CL], F32, kind="ExternalInput").ap()
    wp = nc.dram_tensor("wp", [CL, D], F16, kind="ExternalInput").ap()
    bp = nc.dram_tensor("bp", [D], F32, kind="ExternalInput").ap()
    mp = nc.dram_tensor("mp", [max(n_pat, 1), 128, QCW], F16, kind="ExternalInput").ap()
    y = nc.dram_tensor("y", [T, D], F32, kind="ExternalOutput").ap()

    with tile.TileContext(nc) as tc, nc.allow_low_precision(
        reason="fp16 operands; matmul accumulates fp32 in PSUM"
    ):
        with ExitStack() as octx:
            persist = octx.enter_context(tc.tile_pool(name="persist", bufs=1))
            kT = [persist.tile([128, T], F16, name=f"kT{i}", tag=f"kT{i}") for i in range(NCT)]
            # Q^T natural layout [c, t]; scores matmuls contract K=64 (one
            # head's channels), with even/odd heads on partition halves
            # 0-63 / 64-127 -> disjoint PE row-groups run concurrently.
            qT = [persist.tile([128, T], F16, name=f"qT{i}", tag=f"qT{i}") for i in range(NCT)]
            # per-head stride 128 elements (256B) keeps the AV stationary
            # loads FWL-aligned; col HD of each slot is the ones column
            # that makes the AV matmul emit the softmax sums l on row HD.
            VSW = HL * 128
            vS = [persist.tile([128, VSW], F16, name=f"vS{i}", tag=f"vS{i}") for i in range(NTT)]
            for i in range(NTT):
                nc.gpsimd.memset(vS[i], 0.0)
                nc.gpsimd.memset(
                    vS[i].rearrange("p (h c) -> p h c", c=128)[:, :, HD:HD + 1], 1.0
                )
            attnT = [persist.tile([128, T], F16, name=f"attnT{i}", tag=f"attnT{i}") for i in range(NCT)]

            # resident weights: [128, NDT, CL] with layout (n p) c -> p n c,
            # so [:, dd, sl] is W[dd*128:(dd+1)*128, sl].
            wvr = persist.tile([128, NDT, CL], F16, name="wvr", tag="wvr")
            nc.sync.dma_start(out=wvr, in_=wv.rearrange("(n p) c -> p n c", p=128))
            wqr = persist.tile([128, NDT, CL], F16, name="wqr", tag="wqr")
            nc.sync.dma_start(out=wqr, in_=wq.rearrange("(n p) c -> p n c", p=128))
            wkr = persist.tile([128, NDT, CL], F16, name="wkr", tag="wkr")
            nc.sync.dma_start(out=wkr, in_=wk.rearrange("(n p) c -> p n c", p=128))

            bqs = persist.tile([128, NCT], F32, name="bqs", tag="bqs")
            bks = persist.tile([128, NCT], F32, name="bks", tag="bks")
            nc.sync.dma_start(out=bqs, in_=bq.rearrange("(m p) -> p m", p=128))
            nc.sync.dma_start(out=bks, in_=bk.rearrange("(m p) -> p m", p=128))
            bvb = persist.tile([128, CL], F32, name="bvb", tag="bvb")
            nc.sync.dma_start(
                out=bvb,
                in_=bass.AP(tensor=bv.tensor, offset=bv.offset, ap=[[0, 128]] + list(bv.ap)),
            )
            wps = [persist.tile([128, D], F16, name=f"wps{i}", tag=f"wps{i}") for i in range(NCT)]
            for cc in range(NCT):
                nc.sync.dma_start(out=wps[cc], in_=wp[cc * 128:(cc + 1) * 128, :])
            bpb = persist.tile([128, D], F32, name="bpb", tag="bpb")
            nc.sync.dma_start(
                out=bpb,
                in_=bass.AP(tensor=bp.tensor, offset=bp.offset, ap=[[0, 128]] + list(bp.ap)),
            )
            mts = [persist.tile([128, QCW], F16, name=f"mt{i}", tag=f"mt{i}") for i in range(n_pat)]
            for i in range(n_pat):
                nc.sync.dma_start(out=mts[i], in_=mp[i])

            xtp = octx.enter_context(tc.tile_pool(name="xtp", bufs=2))
            pab = octx.enter_context(tc.tile_pool(name="pab", bufs=2, space="PSUM"))
            pss = octx.enter_context(tc.tile_pool(name="pss", bufs=2, space="PSUM"))
            psav = octx.enter_context(tc.tile_pool(name="psav", bufs=2, space="PSUM"))
            ptl = octx.enter_context(tc.tile_pool(name="ptl", bufs=5))
            utp = octx.enter_context(tc.tile_pool(name="utp", bufs=5))
            ltp = octx.enter_context(tc.tile_pool(name="ltp", bufs=2))
            rvp = octx.enter_context(tc.tile_pool(name="rvp", bufs=2))
            rbp = octx.enter_context(tc.tile_pool(name="rbp", bufs=3))
            ysb = octx.enter_context(tc.tile_pool(name="ysb", bufs=2))
            drp = octx.enter_context(tc.tile_pool(name="drp", bufs=4, space="DRAM"))

            # ---- feeder: QKV compute for one chunk, as small PE items ----
            def issue_xT(ntc):
                xTc = xtp.tile([128, NDT, QCW], F16, name="xTc", tag="xTc")
                for dd in range(NDT):
                    nc.sync.dma_start_transpose(
                        xTc[:, dd, :],
                        x[ntc * QCW:(ntc + 1) * QCW, dd * 128:(dd + 1) * 128],
                    )
                return xTc

            def abc_feeder(ntc):
                """Return a deque of zero-arg callables issuing chunk ntc's
                QKV matmuls in ~2-MM items. x^T transposes issued now."""
                xTc = issue_xT(ntc)
                tsl = slice(ntc * QCW, (ntc + 1) * QCW)
                items = deque()
                for tv in range(TPC):
                    tt = ntc * TPC + tv
                    box = {}
                    for dd0 in range(0, NDT, 2):
                        def v_item(dd0=dd0, box=box, tv=tv, tt=tt):
                            if dd0 == 0:
                                box["pv"] = pab.tile([128, CL], F32, name="pv", tag="pab")
                            for dd in (dd0, dd0 + 1):
                                nc.tensor.matmul(
                                    box["pv"],
                                    lhsT=xTc[:, dd, tv * 128:(tv + 1) * 128],
                                    rhs=wvr[:, dd, :],
                                    start=(dd == 0),
                                    stop=(dd == NDT - 1),
                                )
                            if dd0 == NDT - 2:
                                nc.vector.tensor_add(
                                    vS[tt].rearrange("p (h c) -> p h c", c=128)[:, :, 0:HD],
                                    box["pv"].rearrange("p (h d) -> p h d", h=HL),
                                    bvb.rearrange("p (h d) -> p h d", h=HL),
                                )
                        items.append(v_item)
                for mi in range(2 * NCT):
                    isq = mi < NCT
                    mc = mi % NCT
                    wsrc = wqr if isq else wkr
                    box = {}
                    for dd0 in range(0, NDT, 2):
                        def qk_item(dd0=dd0, box=box, mc=mc, isq=isq, wsrc=wsrc):
                            if dd0 == 0:
                                box["pb"] = pab.tile([128, QCW], F32, name="pb", tag="pab")
                            for dd in (dd0, dd0 + 1):
                                nc.tensor.matmul(
                                    box["pb"],
                                    lhsT=wsrc[:, dd, mc * 128:(mc + 1) * 128],
                                    rhs=xTc[:, dd, :],
                                    start=(dd == 0),
                                    stop=(dd == NDT - 1),
                                )
                            if dd0 == NDT - 2:
                                pb = box["pb"]
                                if isq:
                                    nc.vector.tensor_scalar_add(
                                        qT[mc][:, tsl], pb, bqs[:, mc:mc + 1]
                                    )
                                else:
                                    nc.vector.tensor_scalar_add(
                                        kT[mc][:, tsl], pb, bks[:, mc:mc + 1]
                                    )
                        items.append(qk_item)
                return items

            def proj_feeder(qc):
                """Output projection for q-chunk qc (reads normalized attnT)."""
                items = deque()
                for tv in range(TPC):
                    tt = qc * TPC + tv
                    box = {}
                    for nch in range(NPCH):
                        for cc0 in range(0, NCT, 2):
                            def p_item(cc0=cc0, nch=nch, box=box, tt=tt):
                                if nch == 0 and cc0 == 0:
                                    box["yt"] = ysb.tile([128, D], F32, name="yt", tag="yt")
                                if cc0 == 0:
                                    box["py"] = pab.tile([128, PCH], F32, name="py", tag="pab")
                                for cc in (cc0, cc0 + 1):
                                    nc.tensor.matmul(
                                        box["py"],
                                        lhsT=attnT[cc][:, tt * 128:(tt + 1) * 128],
                                        rhs=wps[cc][:, nch * PCH:(nch + 1) * PCH],
                                        start=(cc == 0),
                                        stop=(cc == NCT - 1),
                                    )
                                if cc0 == NCT - 2:
                                    nc.vector.tensor_add(
                                        box["yt"][:, nch * PCH:(nch + 1) * PCH],
                                        box["py"],
                                        bpb[:, nch * PCH:(nch + 1) * PCH],
                                    )
                                    if nch == NPCH - 1:
                                        nc.sync.dma_start(
                                            out=y[tt * 128:(tt + 1) * 128, :], in_=box["yt"]
                                        )
                            items.append(p_item)
                return items

            abc_q = deque()
            proj_q = deque()

            def pump(n):
                for _ in range(n):
                    if abc_q:
                        abc_q.popleft()()
                    elif proj_q:
                        proj_q.popleft()()
                    else:
                        return

            def drain_abc():
                while abc_q:
                    abc_q.popleft()()

            # ---- flash attention for one q-chunk ----
            # Heads run in even/odd pairs: the two K=64 score matmuls live
            # on disjoint PE row-halves (partitions 0-63 / 64-127) and
            # execute concurrently in the systolic array.
            def flash(qc):
                row = blocks[qc]
                assert row, f"q-chunk {qc} has no active k-tiles"
                pairs = [row[i:i + 2] for i in range(0, len(row), 2)]
                qsl = slice(qc * QCW, (qc + 1) * QCW)
                for hp in range(NCT):
                    pavs = [
                        psav.tile([128, QCW], F32, name="pav", tag="pav")
                        for _ in range(2)
                    ]
                    prev = None
                    for pi, pair in enumerate(pairs):
                        w = len(pair) * QCW
                        pSs = [
                            pss.tile([128, 2 * QCW], F32, name="pS", tag="pS")
                            for _ in range(2)
                        ]
                        for sj, (ki, _) in enumerate(pair):
                            for par in range(2):
                                rsl = slice(par * HD, (par + 1) * HD)
                                nc.tensor.matmul(
                                    pSs[par][:, sj * QCW:(sj + 1) * QCW],
                                    lhsT=kT[hp][rsl, ki * 128:(ki + 1) * 128],
                                    rhs=qT[hp][rsl, qsl],
                                    start=True,
                                    stop=True,
                                )
                        pTs = []
                        for par in range(2):
                            pT = ptl.tile([128, 2 * QCW], F16, name="pT", tag="pT")
                            nc.scalar.activation(pT[:, :w], pSs[par][:, :w], AF.Exp, scale=scale)
                            pTs.append(pT)
                        for sj, (ki, pat) in enumerate(pair):
                            if pat is not None:
                                kind, arg = pat
                                for par in range(2):
                                    sl = pTs[par][:, sj * QCW:(sj + 1) * QCW]
                                    if kind == "tri":
                                        # keep where (q - k) >= 0, else 0
                                        nc.gpsimd.affine_select(
                                            out=sl,
                                            in_=sl,
                                            pattern=[[1, QCW]],
                                            base=arg,
                                            channel_multiplier=-1,
                                            compare_op=mybir.AluOpType.is_ge,
                                            fill=0.0,
                                        )
                                    else:
                                        nc.gpsimd.tensor_mul(sl, sl, mts[arg])
                        # software pipeline: AV for the previous pair issues
                        # after this pair's scores AND a couple of feeder
                        # matmuls, so its exp latency is fully covered.
                        pump(2)
                        if prev is not None:
                            issue_av(hp, pavs, prev, False)
                        prev = (pTs, pair, pi)
                    issue_av(hp, pavs, prev, True)
                    # evacuate unnormalized attn rows + l row; frees banks
                    uts = []
                    for par in range(2):
                        ut = utp.tile([HD + 1, QCW], F16, name="ut", tag="ut")
                        nc.vector.tensor_copy(ut, pavs[par][0:HD + 1, :])
                        uts.append(ut)
                    hp_norm(qc, hp, uts)
                    pump(2)

            def issue_av(hp, pavs, prev, is_last):
                pTs, pair, pi = prev
                for par in range(2):
                    h = 2 * hp + par
                    for sj, (ki, _) in enumerate(pair):
                        nc.tensor.matmul(
                            pavs[par],
                            lhsT=vS[ki][:, h * 128:h * 128 + 128],
                            rhs=pTs[par][:, sj * QCW:(sj + 1) * QCW],
                            start=(pi == 0 and sj == 0),
                            stop=(is_last and sj == len(pair) - 1),
                        )

            def hp_norm(qc, hp, uts):
                """1/l for one head pair. The two l rows bounce through DRAM
                so the reciprocal can run on a [64, 16] reshape (reciprocal
                cost is free-dim-bound: 16 elems/lane, not 512), then
                broadcast back as [64, QCW] and multiply into attnT."""
                lb = drp.tile([2, QCW], F16, name="lb", tag="lb")
                for par in range(2):
                    nc.sync.dma_start(out=lb[par:par + 1, :], in_=uts[par][HD:HD + 1, :])
                nfd = 2 * QCW // 64
                lsb = ltp.tile([64, nfd], F16, name="lsb", tag="lsb")
                nc.sync.dma_start(
                    out=lsb,
                    in_=bass.AP(tensor=lb.tensor, offset=lb.offset, ap=[[nfd, 64], [1, nfd]]),
                )
                rinv = rvp.tile([64, nfd], F16, name="rinv", tag="rinv")
                nc.vector.reciprocal(rinv, lsb)
                scr = drp.tile([2, QCW], F16, name="scr", tag="scr")
                nc.sync.dma_start(
                    out=bass.AP(tensor=scr.tensor, offset=scr.offset, ap=[[nfd, 64], [1, nfd]]),
                    in_=rinv,
                )
                for par in range(2):
                    row = scr[par:par + 1, :]
                    rbs = rbp.tile([HD, QCW], F16, name="rbs", tag="rbs")
                    nc.sync.dma_start(
                        out=rbs,
                        in_=bass.AP(tensor=row.tensor, offset=row.offset, ap=[[0, HD]] + list(row.ap)[1:]),
                    )
                    nc.vector.tensor_mul(
                        attnT[hp][par * HD:(par + 1) * HD, qc * QCW:(qc + 1) * QCW],
                        uts[par][0:HD, :],
                        rbs,
                    )

            # ---- fused main loop ----
            abc_q.extend(abc_feeder(0))
            drain_abc()
            for ntc in range(NQC):
                if ntc + 1 < NQC:
                    abc_q.extend(abc_feeder(ntc + 1))
                flash(ntc)
                drain_abc()
                proj_q.extend(proj_feeder(ntc))
            while proj_q:
                proj_q.popleft()()
    nc.compile()
    return nc


def classify_mask(mask_bool, T):
    """Classify S^T blocks [k-tile 128, q-chunk 512] as skip / full / mixed.

    mask_bool: [T, T] bool, mask_bool[q, k] = attend(q -> k).
    Returns (blocks, patterns): blocks[qc] = list of (ki, pat_idx|None),
    patterns = np.ndarray [n_pat, 128, QCW] float32.
    """
    QCW = min(512, T)
    NQC = T // QCW
    NKT = T // 128
    maskT = mask_bool.T  # [k, q]
    patterns = []
    pat_index = {}
    blocks = []
    for qc in range(NQC):
        row = []
        for ki in range(NKT):
            blk = maskT[ki * 128:(ki + 1) * 128, qc * QCW:(qc + 1) * QCW]
            if not blk.any():
                continue
            if blk.all():
                row.append((ki, None))
                continue
            # tril-offset block? keep iff k <= q, i.e. p <= base + f
            base = qc * QCW - ki * 128
            p = np.arange(128)[:, None]
            f = np.arange(QCW)[None, :]
            if np.array_equal(blk, p <= base + f):
                row.append((ki, ("tri", base)))
                continue
            key = blk.tobytes()
            if key not in pat_index:
                pat_index[key] = len(patterns)
                patterns.append(blk.astype(np.float32))
            row.append((ki, ("pat", pat_index[key])))
        blocks.append(row)
    n_pat = len(patterns)
    if patterns:
        pats = np.stack(patterns)
    else:
        pats = np.zeros((1, 128, QCW), np.float32)
    return blocks, pats, n_pat


_prog_cache = {}


def _get_program(T, D, HL, mask_bool):
    key = (T, D, HL, mask_bool.tobytes())
    if key not in _prog_cache:
        blocks, pats, n_pat = classify_mask(mask_bool, T)
        nc = build_program(T, D, HL, n_pat, blocks)
        _prog_cache[key] = (nc, blocks, pats)
    return _prog_cache[key]


def kernel(x, W_qkv, b_qkv, W_proj, b_proj, mask):
    out, _ = run_attention(x, W_qkv, b_qkv, W_proj, b_proj, mask)
    return out


def run_attention(x, W_qkv, b_qkv, W_proj, b_proj, mask, trace=False):
    x = np.ascontiguousarray(np.asarray(x, dtype=np.float32))
    W_qkv = np.asarray(W_qkv, dtype=np.float32)
    b_qkv = np.asarray(b_qkv, dtype=np.float32)
    W_proj = np.asarray(W_proj, dtype=np.float32)
    b_proj = np.asarray(b_proj, dtype=np.float32)
    Bc, T, D = x.shape
    NH = NH_FULL
    HL = NH // 2  # heads per core (two head-groups)
    CL = HL * HD

    mask_bool = np.asarray(mask)[0, 0] != 0

    nc, blocks, pats = _get_program(T, D, HL, mask_bool)

    in_maps = []
    n_cores = 2 * Bc
    for c in range(n_cores):
        b, g = c // 2, c % 2
        sl = slice(g * CL, (g + 1) * CL)
        in_maps.append({
            "x": np.ascontiguousarray(x[b]).astype(np.float16),
            "wq": np.ascontiguousarray(W_qkv[:, 0 * D:1 * D][:, sl]).astype(np.float16),
            "wk": np.ascontiguousarray(W_qkv[:, 1 * D:2 * D][:, sl]).astype(np.float16),
            "wv": np.ascontiguousarray(W_qkv[:, 2 * D:3 * D][:, sl]).astype(np.float16),
            "bq": np.ascontiguousarray(b_qkv[0 * D:1 * D][sl]),
            "bk": np.ascontiguousarray(b_qkv[1 * D:2 * D][sl]),
            "bv": np.ascontiguousarray(b_qkv[2 * D:3 * D][sl]),
            "wp": np.ascontiguousarray(W_proj[sl, :]).astype(np.float16),
            "bp": b_proj if g == 0 else np.zeros_like(b_proj),
            "mp": pats.astype(np.float16),
        })

    res = run_bass_kernel_spmd(nc, in_maps, list(range(n_cores)), trace=trace)
    out = np.empty((Bc, T, D), np.float32)
    for b in range(Bc):
        out[b] = res.results[2 * b]["y"] + res.results[2 * b + 1]["y"]
    return out, res


# revision 15
# speedup vs baseline: 1.3918x; 1.0133x over previous
"""Multi-head causal attention (B=4, T=2048, D=1024, 16 heads) on 8 TRN2 cores.

Sharding: core c -> batch b = c//2, head-group g = c%2 (8 of 16 heads).
Each core computes its batch's QKV for its heads, flash-style causal
attention with scores kept transposed (S^T[k, q]) so softmax sums come
free via a ones-column appended to V, then a partial output projection
y_part = attn_local @ W_proj[rows]. Host sums the two head-group partials
per batch.

Schedule: a single fused loop over 512-wide t-chunks. Chunk ntc's QKV
matmuls run, then flash attention for q-chunk ntc (which needs K/V
chunks 0..ntc only, all available). The next chunk's QKV matmuls and the
previous chunk's output-projection matmuls are spliced into the
exp-latency gaps of the flash loop so the PE never idles waiting on the
scalar engine. Softmax normalization is deferred: per head the
unnormalized attention rows and the l-row are evacuated to SBUF
immediately (releasing PSUM), then one batched reciprocal per q-chunk +
a DRAM-bounce partition-broadcast produce 1/l, and the normalize
multiplies run on the vector engine underneath the next chunk's flash.

Matmul operands are fp16; accumulation stays fp32 in PSUM.
"""

import math
from collections import deque
from contextlib import ExitStack

import numpy as np

import concourse.bacc as bacc
import concourse.bass as bass
import concourse.mybir as mybir
import concourse.tile as tile
from concourse.bass_utils import run_bass_kernel_spmd

AF = mybir.ActivationFunctionType
F32 = mybir.dt.float32
F16 = mybir.dt.float16

B_FULL = 4
T_FULL = 2048
D_FULL = 1024
NH_FULL = 16
HD = 64


def build_program(T, D, HL, n_pat, blocks):
    """Build the per-core SPMD program.

    T: sequence length, D: model dim, HL: local heads, n_pat: number of
    distinct mixed-mask pattern tiles, blocks: per q-chunk list of
    (k_tile_index, pattern_index_or_None) for active score blocks.
    """
    CL = HL * HD            # local channels (q, k, or v width)
    NDT = D // 128          # d-tiles (contraction tiles for qkv matmuls)
    NTT = T // 128          # t-tiles
    QCW = min(512, T)       # q-chunk width
    NQC = T // QCW
    TPC = QCW // 128        # t-tiles per q-chunk
    NCT = CL // 128         # c-tiles for q/k/attn storage
    PCH = min(512, D)       # proj output chunk
    NPCH = D // PCH
    scale = 1.0 / math.sqrt(HD)

    nc = bacc.Bacc("TRN2", target_bir_lowering=False, debug=False)
    x = nc.dram_tensor("x", [T, D], F16, kind="ExternalInput").ap()
    wq = nc.dram_tensor("wq", [D, CL], F16, kind="ExternalInput").ap()
    wk = nc.dram_tensor("wk", [D, CL], F16, kind="ExternalInput").ap()
    wv = nc.dram_tensor("wv", [D, CL], F16, kind="ExternalInput").ap()
    bq = nc.dram_tensor("bq", [CL], F32, kind="ExternalInput").ap()
    bk = nc.dram_tensor("bk", [CL], F32, kind="ExternalInput").ap()
    bv = nc.dram_tensor("bv", [CL], F32, kind="ExternalInput").ap()
    wp = nc.dram_tensor("wp", [CL, D], F16, kind="ExternalInput").ap()
    bp = nc.dram_tensor("bp", [D], F32, kind="ExternalInput").ap()
    mp = nc.dram_tensor("mp", [max(n_pat, 1), 128, QCW], F16, kind="ExternalInput").ap()
    y = nc.dram_tensor("y", [T, D], F32, kind="ExternalOutput").ap()

    with tile.TileContext(nc) as tc, nc.allow_low_precision(
        reason="fp16 operands; matmul accumulates fp32 in PSUM"
    ):
        with ExitStack() as octx:
            persist = octx.enter_context(tc.tile_pool(name="persist", bufs=1))
            kT = [persist.tile([128, T], F16, name=f"kT{i}", tag=f"kT{i}") for i in range(NCT)]
            # Q^T natural layout [c, t]; scores matmuls contract K=64 (one
            # head's channels), with even/odd heads on partition halves
            # 0-63 / 64-127 -> disjoint PE row-groups run concurrently.
            qT = [persist.tile([128, T], F16, name=f"qT{i}", tag=f"qT{i}") for i in range(NCT)]
            # per-head stride 128 elements (256B) keeps the AV stationary
            # loads FWL-aligned; col HD of each slot is the ones column
            # that makes the AV matmul emit the softmax sums l on row HD.
            VSW = HL * 128
            vS = [persist.tile([128, VSW], F16, name=f"vS{i}", tag=f"vS{i}") for i in range(NTT)]
            for i in range(NTT):
                nc.gpsimd.memset(vS[i], 0.0)
                nc.gpsimd.memset(
                    vS[i].rearrange("p (h c) -> p h c", c=128)[:, :, HD:HD + 1], 1.0
                )
            attnT = [persist.tile([128, T], F16, name=f"attnT{i}", tag=f"attnT{i}") for i in range(NCT)]

            # resident weights: [128, NDT, CL] with layout (n p) c -> p n c,
            # so [:, dd, sl] is W[dd*128:(dd+1)*128, sl].
            wvr = persist.tile([128, NDT, CL], F16, name="wvr", tag="wvr")
            nc.sync.dma_start(out=wvr, in_=wv.rearrange("(n p) c -> p n c", p=128))
            wqr = persist.tile([128, NDT, CL], F16, name="wqr", tag="wqr")
            nc.sync.dma_start(out=wqr, in_=wq.rearrange("(n p) c -> p n c", p=128))
            wkr = persist.tile([128, NDT, CL], F16, name="wkr", tag="wkr")
            nc.sync.dma_start(out=wkr, in_=wk.rearrange("(n p) c -> p n c", p=128))

            bqs = persist.tile([128, NCT], F32, name="bqs", tag="bqs")
            bks = persist.tile([128, NCT], F32, name="bks", tag="bks")
            nc.sync.dma_start(out=bqs, in_=bq.rearrange("(m p) -> p m", p=128))
            nc.sync.dma_start(out=bks, in_=bk.rearrange("(m p) -> p m", p=128))
            bvb = persist.tile([128, CL], F32, name="bvb", tag="bvb")
            nc.sync.dma_start(
                out=bvb,
                in_=bass.AP(tensor=bv.tensor, offset=bv.offset, ap=[[0, 128]] + list(bv.ap)),
            )
            wps = [persist.tile([128, D], F16, name=f"wps{i}", tag=f"wps{i}") for i in range(NCT)]
            for cc in range(NCT):
                nc.sync.dma_start(out=wps[cc], in_=wp[cc * 128:(cc + 1) * 128, :])
            bpb = persist.tile([128, D], F32, name="bpb", tag="bpb")
            nc.sync.dma_start(
                out=bpb,
                in_=bass.AP(tensor=bp.tensor, offset=bp.offset, ap=[[0, 128]] + list(bp.ap)),
            )
            mts = [persist.tile([128, QCW], F16, name=f"mt{i}", tag=f"mt{i}") for i in range(n_pat)]
            for i in range(n_pat):
                nc.sync.dma_start(out=mts[i], in_=mp[i])

            xtp = octx.enter_context(tc.tile_pool(name="xtp", bufs=2))
            pab = octx.enter_context(tc.tile_pool(name="pab", bufs=2, space="PSUM"))
            pss = octx.enter_context(tc.tile_pool(name="pss", bufs=2, space="PSUM"))
            psav = octx.enter_context(tc.tile_pool(name="psav", bufs=2, space="PSUM"))
            ptl = octx.enter_context(tc.tile_pool(name="ptl", bufs=5))
            utp = octx.enter_context(tc.tile_pool(name="utp", bufs=5))
            ltp = octx.enter_context(tc.tile_pool(name="ltp", bufs=2))
            rvp = octx.enter_context(tc.tile_pool(name="rvp", bufs=2))
            rbp = octx.enter_context(tc.tile_pool(name="rbp", bufs=3))
            ysb = octx.enter_context(tc.tile_pool(name="ysb", bufs=2))
            drp = octx.enter_context(tc.tile_pool(name="drp", bufs=4, space="DRAM"))

            # ---- feeder: QKV compute for one chunk, as small PE items ----
            def issue_xT(ntc):
                xTc = xtp.tile([128, NDT, QCW], F16, name="xTc", tag="xTc")
                for dd in range(NDT):
                    nc.sync.dma_start_transpose(
                        xTc[:, dd, :],
                        x[ntc * QCW:(ntc + 1) * QCW, dd * 128:(dd + 1) * 128],
                    )
                return xTc

            def abc_feeder(ntc):
                """Return a deque of zero-arg callables issuing chunk ntc's
                QKV matmuls in ~2-MM items. x^T transposes issued now."""
                xTc = issue_xT(ntc)
                tsl = slice(ntc * QCW, (ntc + 1) * QCW)
                items = deque()
                for tv in range(TPC):
                    tt = ntc * TPC + tv
                    box = {}
                    for dd0 in range(0, NDT, 2):
                        def v_item(dd0=dd0, box=box, tv=tv, tt=tt):
                            if dd0 == 0:
                                box["pv"] = pab.tile([128, CL], F32, name="pv", tag="pab")
                            for dd in (dd0, dd0 + 1):
                                nc.tensor.matmul(
                                    box["pv"],
                                    lhsT=xTc[:, dd, tv * 128:(tv + 1) * 128],
                                    rhs=wvr[:, dd, :],
                                    start=(dd == 0),
                                    stop=(dd == NDT - 1),
                                )
                            if dd0 == NDT - 2:
                                nc.vector.tensor_add(
                                    vS[tt].rearrange("p (h c) -> p h c", c=128)[:, :, 0:HD],
                                    box["pv"].rearrange("p (h d) -> p h d", h=HL),
                                    bvb.rearrange("p (h d) -> p h d", h=HL),
                                )
                        items.append(v_item)
                for mi in range(2 * NCT):
                    isq = mi < NCT
                    mc = mi % NCT
                    wsrc = wqr if isq else wkr
                    box = {}
                    for dd0 in range(0, NDT, 2):
                        def qk_item(dd0=dd0, box=box, mc=mc, isq=isq, wsrc=wsrc):
                            if dd0 == 0:
                                box["pb"] = pab.tile([128, QCW], F32, name="pb", tag="pab")
                            for dd in (dd0, dd0 + 1):
                                nc.tensor.matmul(
                                    box["pb"],
                                    lhsT=wsrc[:, dd, mc * 128:(mc + 1) * 128],
                                    rhs=xTc[:, dd, :],
                                    start=(dd == 0),
                                    stop=(dd == NDT - 1),
                                )
                            if dd0 == NDT - 2:
                                pb = box["pb"]
                                if isq:
                                    nc.vector.tensor_scalar_add(
                                        qT[mc][:, tsl], pb, bqs[:, mc:mc + 1]
                                    )
                                else:
                                    nc.vector.tensor_scalar_add(
                                        kT[mc][:, tsl], pb, bks[:, mc:mc + 1]
                                    )
                        items.append(qk_item)
                return items

            def proj_feeder(qc):
                """Output projection for q-chunk qc (reads normalized attnT)."""
                items = deque()
                for tv in range(TPC):
                    tt = qc * TPC + tv
                    box = {}
                    for nch in range(NPCH):
                        for cc0 in range(0, NCT, 2):
                            def p_item(cc0=cc0, nch=nch, box=box, tt=tt):
                                if nch == 0 and cc0 == 0:
                                    box["yt"] = ysb.tile([128, D], F32, name="yt", tag="yt")
                                if cc0 == 0:
                                    box["py"] = pab.tile([128, PCH], F32, name="py", tag="pab")
                                for cc in (cc0, cc0 + 1):
                                    nc.tensor.matmul(
                                        box["py"],
                                        lhsT=attnT[cc][:, tt * 128:(tt + 1) * 128],
                                        rhs=wps[cc][:, nch * PCH:(nch + 1) * PCH],
                                        start=(cc == 0),
                                        stop=(cc == NCT - 1),
                                    )
                                if cc0 == NCT - 2:
                                    nc.vector.tensor_add(
                                        box["yt"][:, nch * PCH:(nch + 1) * PCH],
                                        box["py"],
                                        bpb[:, nch * PCH:(nch + 1) * PCH],
                                    )
                                    if nch == NPCH - 1:
                                        nc.sync.dma_start(
                                            out=y[tt * 128:(tt + 1) * 128, :], in_=box["yt"]
                                        )
                            items.append(p_item)
                return items

            abc_q = deque()
            proj_q = deque()

            def pump(n):
                for _ in range(n):
                    if abc_q:
                        abc_q.popleft()()
                    elif proj_q:
                        proj_q.popleft()()
                    else:
                        return

            def drain_abc():
                while abc_q:
                    abc_q.popleft()()

            # ---- flash attention for one q-chunk ----
            # Heads run in even/odd pairs: the two K=64 score matmuls live
            # on disjoint PE row-halves (partitions 0-63 / 64-127) and
            # execute concurrently in the systolic array.
            def flash(qc):
                row = blocks[qc]
                assert row, f"q-chunk {qc} has no active k-tiles"
                qsl = slice(qc * QCW, (qc + 1) * QCW)
                for hp in range(NCT):
                    pavs = [
                        psav.tile([128, QCW], F32, name="pav", tag="pav")
                        for _ in range(2)
                    ]
                    prev = None
                    for bi, (ki, pat) in enumerate(row):
                        # both parities' S^T blocks for this k-tile share one
                        # pS tile (cols 0:512 even head, 512:1024 odd head):
                        # one exp serves both, and the two K=64 matmuls are
                        # issued back-to-back so they run concurrently.
                        pS = pss.tile([128, 2 * QCW], F32, name="pS", tag="pS")
                        for par in range(2):
                            rsl = slice(par * HD, (par + 1) * HD)
                            nc.tensor.matmul(
                                pS[:, par * QCW:(par + 1) * QCW],
                                lhsT=kT[hp][rsl, ki * 128:(ki + 1) * 128],
                                rhs=qT[hp][rsl, qsl],
                                start=True,
                                stop=True,
                            )
                        pT = ptl.tile([128, 2 * QCW], F16, name="pT", tag="pT")
                        nc.scalar.activation(pT, pS, AF.Exp, scale=scale)
                        if pat is not None:
                            kind, arg = pat
                            for par in range(2):
                                sl = pT[:, par * QCW:(par + 1) * QCW]
                                if kind == "tri":
                                    # keep where (q - k) >= 0, else 0
                                    nc.gpsimd.affine_select(
                                        out=sl,
                                        in_=sl,
                                        pattern=[[1, QCW]],
                                        base=arg,
                                        channel_multiplier=-1,
                                        compare_op=mybir.AluOpType.is_ge,
                                        fill=0.0,
                                    )
                                else:
                                    nc.gpsimd.tensor_mul(sl, sl, mts[arg])
                        # software pipeline: AV for the previous k-tile issues
                        # after this k-tile's scores AND a feeder matmul, so
                        # its exp latency is fully covered.
                        pump(1)
                        if prev is not None:
                            issue_av(hp, pavs, prev, False)
                        prev = (pT, ki, bi)
                    issue_av(hp, pavs, prev, True)
                    # evacuate unnormalized attn rows + l row; frees banks
                    uts = []
                    for par in range(2):
                        ut = utp.tile([HD + 1, QCW], F16, name="ut", tag="ut")
                        nc.vector.tensor_copy(ut, pavs[par][0:HD + 1, :])
                        uts.append(ut)
                    hp_norm(qc, hp, uts)
                    pump(2)

            def issue_av(hp, pavs, prev, is_last):
                pT, ki, bi = prev
                for par in range(2):
                    h = 2 * hp + par
                    nc.tensor.matmul(
                        pavs[par],
                        lhsT=vS[ki][:, h * 128:h * 128 + 128],
                        rhs=pT[:, par * QCW:(par + 1) * QCW],
                        start=(bi == 0),
                        stop=is_last,
                    )

            def hp_norm(qc, hp, uts):
                """1/l for one head pair. The two l rows bounce through DRAM
                so the reciprocal can run on a [64, 16] reshape (reciprocal
                cost is free-dim-bound: 16 elems/lane, not 512), then
                broadcast back as [64, QCW] and multiply into attnT."""
                lb = drp.tile([2, QCW], F16, name="lb", tag="lb")
                for par in range(2):
                    nc.sync.dma_start(out=lb[par:par + 1, :], in_=uts[par][HD:HD + 1, :])
                nfd = 2 * QCW // 64
                lsb = ltp.tile([64, nfd], F16, name="lsb", tag="lsb")
                nc.sync.dma_start(
                    out=lsb,
                    in_=bass.AP(tensor=lb.tensor, offset=lb.offset, ap=[[nfd, 64], [1, nfd]]),
                )
                rinv = rvp.tile([64, nfd], F16, name="rinv", tag="rinv")
                nc.vector.reciprocal(rinv, lsb)
                scr = drp.tile([2, QCW], F16, name="scr", tag="scr")
                nc.sync.dma_start(
                    out=bass.AP(tensor=scr.tensor, offset=scr.offset, ap=[[nfd, 64], [1, nfd]]),
                    in_=rinv,
                )
                for par in range(2):
                    row = scr[par:par + 1, :]
                    rbs = rbp.tile([HD, QCW], F16, name="rbs", tag="rbs")
                    nc.sync.dma_start(
                        out=rbs,
                        in_=bass.AP(tensor=row.tensor, offset=row.offset, ap=[[0, HD]] + list(row.ap)[1:]),
                    )
                    nc.vector.tensor_mul(
                        attnT[hp][par * HD:(par + 1) * HD, qc * QCW:(qc + 1) * QCW],
                        uts[par][0:HD, :],
                        rbs,
                    )

            # ---- fused main loop ----
            abc_q.extend(abc_feeder(0))
            drain_abc()
            for ntc in range(NQC):
                if ntc + 1 < NQC:
                    abc_q.extend(abc_feeder(ntc + 1))
                flash(ntc)
                drain_abc()
                proj_q.extend(proj_feeder(ntc))
            while proj_q:
                proj_q.popleft()()
    nc.compile()
    return nc


def classify_mask(mask_bool, T):
    """Classify S^T blocks [k-tile 128, q-chunk 512] as skip / full / mixed.

    mask_bool: [T, T] bool, mask_bool[q, k] = attend(q -> k).
    Returns (blocks, patterns): blocks[qc] = list of (ki, pat_idx|None),
    patterns = np.ndarray [n_pat, 128, QCW] float32.
    """
    QCW = min(512, T)
    NQC = T // QCW
    NKT = T // 128
    maskT = mask_bool.T  # [k, q]
    patterns = []
    pat_index = {}
    blocks = []
    for qc in range(NQC):
        row = []
        for ki in range(NKT):
            blk = maskT[ki * 128:(ki + 1) * 128, qc * QCW:(qc + 1) * QCW]
            if not blk.any():
                continue
            if blk.all():
                row.append((ki, None))
                continue
            # tril-offset block? keep iff k <= q, i.e. p <= base + f
            base = qc * QCW - ki * 128
            p = np.arange(128)[:, None]
            f = np.arange(QCW)[None, :]
            if np.array_equal(blk, p <= base + f):
                row.append((ki, ("tri", base)))
                continue
            key = blk.tobytes()
            if key not in pat_index:
                pat_index[key] = len(patterns)
                patterns.append(blk.astype(np.float32))
            row.append((ki, ("pat", pat_index[key])))
        blocks.append(row)
    n_pat = len(patterns)
    if patterns:
        pats = np.stack(patterns)
    else:
        pats = np.zeros((1, 128, QCW), np.float32)
    return blocks, pats, n_pat


_prog_cache = {}


def _get_program(T, D, HL, mask_bool):
    key = (T, D, HL, mask_bool.tobytes())
    if key not in _prog_cache:
        blocks, pats, n_pat = classify_mask(mask_bool, T)
        nc = build_program(T, D, HL, n_pat, blocks)
        _prog_cache[key] = (nc, blocks, pats)
    return _prog_cache[key]


def kernel(x, W_qkv, b_qkv, W_proj, b_proj, mask):
    out, _ = run_attention(x, W_qkv, b_qkv, W_proj, b_proj, mask)
    return out


def run_attention(x, W_qkv, b_qkv, W_proj, b_proj, mask, trace=False):
    x = np.ascontiguousarray(np.asarray(x, dtype=np.float32))
    W_qkv = np.asarray(W_qkv, dtype=np.float32)
    b_qkv = np.asarray(b_qkv, dtype=np.float32)
    W_proj = np.asarray(W_proj, dtype=np.float32)
    b_proj = np.asarray(b_proj, dtype=np.float32)
    Bc, T, D = x.shape
    NH = NH_FULL
    HL = NH // 2  # heads per core (two head-groups)
    CL = HL * HD

    mask_bool = np.asarray(mask)[0, 0] != 0

    nc, blocks, pats = _get_program(T, D, HL, mask_bool)

    in_maps = []
    n_cores = 2 * Bc
    for c in range(n_cores):
        b, g = c // 2, c % 2
        sl = slice(g * CL, (g + 1) * CL)
        in_maps.append({
            "x": np.ascontiguousarray(x[b]).astype(np.float16),
            "wq": np.ascontiguousarray(W_qkv[:, 0 * D:1 * D][:, sl]).astype(np.float16),
            "wk": np.ascontiguousarray(W_qkv[:, 1 * D:2 * D][:, sl]).astype(np.float16),
            "wv": np.ascontiguousarray(W_qkv[:, 2 * D:3 * D][:, sl]).astype(np.float16),
            "bq": np.ascontiguousarray(b_qkv[0 * D:1 * D][sl]),
            "bk": np.ascontiguousarray(b_qkv[1 * D:2 * D][sl]),
            "bv": np.ascontiguousarray(b_qkv[2 * D:3 * D][sl]),
            "wp": np.ascontiguousarray(W_proj[sl, :]).astype(np.float16),
            "bp": b_proj if g == 0 else np.zeros_like(b_proj),
            "mp": pats.astype(np.float16),
        })

    res = run_bass_kernel_spmd(nc, in_maps, list(range(n_cores)), trace=trace)
    out = np.empty((Bc, T, D), np.float32)
    for b in range(Bc):
        out[b] = res.results[2 * b]["y"] + res.results[2 * b + 1]["y"]
    return out, res
